# revision 1
# baseline (speedup 1.0000x reference)
"""Trainium2 Bass kernel for 3x3 VALID conv: x[32,128,64,64] * w[256,128,3,3] + bias.

Strategy:
  - Data-parallel over batch: 8 cores x 4 images each; weights/bias replicated.
  - Per core: implicit GEMM. Contraction dim = C_IN = 128 = partition dim.
    For each filter tap (u,v), accumulate
        psum[o, p] += W[c, o; u,v].T @ x[c, p + u*64 + v]
    over the flattened output grid of 62 rows x 64 cols (contiguous moving
    operand; the last 2 cols of each row are invalid and trimmed on host).
  - float32r matmuls (1 cycle/row for free-dim >= 256 vs 4 cycles/row fp32).
  - Critical-path-first DMA: w tap0 + first 17 input rows land first; weights
    stream on the ScalarE HWDGE ring in parallel with inputs on the Sync ring.
  - PSUM -> SBUF evacuation + bias add on VectorE; one output DMA per chunk.
"""

import numpy as np

import concourse.bacc as bacc
import concourse.tile as tile
from concourse import mybir
from concourse.bass_utils import run_bass_kernel_spmd

N_CORES = 8
B_FULL, C_IN, H, W = 32, 128, 64, 64
C_OUT, KH, KW = 256, 3, 3
B_LOC = B_FULL // N_CORES          # images per core
H_OUT, W_OUT = H - KH + 1, W - KW + 1   # 62, 62
N_HALF = C_OUT // 128              # 2 output-channel halves
ROWS_PER_CHUNK = 8                 # 8 out rows x 64 cols = 512 = one PSUM bank
X_PAD = (H_OUT + KH - 1) * W + 128  # padded free size so shifted reads stay in-bounds
X_PIECES = [(0, 656), (656, 1824), (1824, 2976), (2976, H * W)]

_cached = {}


def _build_nc():
    f32 = mybir.dt.float32
    f32r = mybir.dt.float32r
    nc = bacc.Bacc()

    x_d = nc.declare_dram_parameter("x", [B_LOC, C_IN, H, W], f32r, isOutput=False)
    w_d = nc.declare_dram_parameter(
        "w", [C_IN, N_HALF, KH * KW, 128], f32r, isOutput=False
    )
    b_d = nc.declare_dram_parameter("bias_in", [128, N_HALF], f32, isOutput=False)
    y_d = nc.declare_dram_parameter(
        "y", [B_LOC, N_HALF, 128, H_OUT, W], f32, isOutput=True
    )

    n_chunks = (H_OUT + ROWS_PER_CHUNK - 1) // ROWS_PER_CHUNK
    with tile.TileContext(nc) as tc:
        with (
            tc.tile_pool(name="const", bufs=1) as cpool,
            tc.tile_pool(name="xin", bufs=2) as xpool,
            tc.tile_pool(name="out", bufs=4) as opool,
            tc.tile_pool(name="psum", bufs=4, space="PSUM") as ppool,
        ):
            w_t = cpool.tile([C_IN, N_HALF, KH * KW, 128], f32r)
            b_t = cpool.tile([128, N_HALF], f32)

            # Bias rides the (otherwise idle) ScalarE HWDGE ring.
            nc.scalar.dma_start(b_t[:], b_d[:])
            # Critical path for the first matmul group: w half0 taps 0-2, then
            # x piece0 on the Sync HWDGE ring. The [c, half, uv, o] host layout
            # keeps every tap subrange contiguous per partition, so the first
            # three matmuls can start while the remaining taps stream in.
            nc.sync.dma_start(w_t[:, 0, 0:3], w_d[:, 0, 0:3])

            def load_x(b, first):
                x_t = xpool.tile([C_IN, X_PAD], f32r, tag="x")
                x_flat = x_d[b].rearrange("c h w -> c (h w)")
                for k, (lo, hi) in enumerate(X_PIECES):
                    nc.sync.dma_start(x_t[:, lo:hi], x_flat[:, lo:hi])
                    if first and k == 0:
                        nc.sync.dma_start(w_t[:, 0, 3 : KH * KW], w_d[:, 0, 3 : KH * KW])
                        nc.sync.dma_start(w_t[:, 1], w_d[:, 1])
                # Tail pad: (arbitrary) real data — feeds only the invalid
                # output columns (j >= 62) that the host trims away.
                nc.sync.dma_start(
                    x_t[:, H * W : X_PAD], x_flat[:, 0 : X_PAD - H * W]
                )
                return x_t

            for b in range(B_LOC):
                x_t = load_x(b, first=(b == 0))
                for chunk in range(n_chunks):
                    i0 = chunk * ROWS_PER_CHUNK
                    r = min(ROWS_PER_CHUNK, H_OUT - i0)
                    n = r * W
                    p0 = i0 * W
                    for half in range(N_HALF):
                        ps = ppool.tile([128, ROWS_PER_CHUNK, W], f32, tag="ps")
                        for uv in range(KH * KW):
                            u, v = divmod(uv, KW)
                            shift = p0 + u * W + v
                            nc.tensor.matmul(
                                ps[:, 0:r, :],
                                w_t[:, half, uv, :],
                                x_t[:, shift : shift + n],
                                start=(uv == 0),
                                stop=(uv == KH * KW - 1),
                            )
                        o_t = opool.tile([128, ROWS_PER_CHUNK, W], f32, tag="o")
                        nc.vector.tensor_scalar_add(
                            o_t[:, 0:r, :], ps[:, 0:r, :], b_t[:, half : half + 1]
                        )
                        nc.sync.dma_start(
                            y_d[b, half, :, i0 : i0 + r, :], o_t[:, 0:r, :]
                        )

    nc.compile()
    if not nc.is_finalized():
        nc.finalize()
    return nc


def kernel(inputs, weights, bias, profile=False, trace_kwargs=None):
    inputs = np.ascontiguousarray(inputs, dtype=np.float32)
    # [O, C, KH, KW] -> [C, half, KH*KW, o_local]  (lhsT layout: contraction dim
    # on partitions; each half contiguous per partition for fast DMA)
    w_t = np.ascontiguousarray(
        weights.astype(np.float32)
        .reshape(N_HALF, 128, C_IN, KH * KW)
        .transpose(2, 0, 3, 1)
    )
    # [C_OUT, 1] -> [128, N_HALF] with bias_sb[p, h] = bias[h*128 + p]
    b_t = np.ascontiguousarray(
        bias.astype(np.float32).reshape(N_HALF, 128).T
    )

    if "nc" not in _cached:
        _cached["nc"] = _build_nc()
    nc = _cached["nc"]

    in_maps = [
        {
            "x": inputs[i * B_LOC : (i + 1) * B_LOC],
            "w": w_t,
            "bias_in": b_t,
        }
        for i in range(N_CORES)
    ]
    res = run_bass_kernel_spmd(
        nc,
        in_maps,
        list(range(N_CORES)),
        trace=profile,
        **(trace_kwargs or {}),
    )
    _cached["last_result"] = res

    shards = []
    for i in range(N_CORES):
        y = res.results[i]["y"]  # [B_LOC, 2, 128, 62, 64]
        shards.append(y.reshape(B_LOC, C_OUT, H_OUT, W)[..., :W_OUT])
    return np.ascontiguousarray(np.concatenate(shards, axis=0), dtype=np.float32)



# revision 2
# speedup vs baseline: 1.0891x; 1.0891x over previous
"""Trainium2 Bass kernel for 3x3 VALID conv: x[32,128,64,64] * w[256,128,3,3] + bias.

Strategy (v2):
  - Data-parallel over batch: 8 cores x 4 images each; weights/bias replicated.
  - Per core: implicit GEMM in bf16. Contraction dim = C_IN = 128 = partition
    dim. For each filter tap (u,v), accumulate
        psum[o, i, j] += W[c, o; u,v].T @ x[c, i+u, j+v]
    over [r rows x 62 cols] chunks via a strided 2D moving-operand AP (row
    stride 64), so no wasted columns and no padding.
  - bf16 operands: LDWEIGHTS is a separate instruction the PE pulls ahead of
    in-flight matmuls (and FWL halves its cost), so weight loads hide under
    the 496-cycle matmul stream. fp32r pays a partially-serialized 4-byte
    weight load (~64ns/MM extra).
  - PSUM -> SBUF evacuation + bias add on VectorE; one output DMA per chunk.
"""

import numpy as np
import ml_dtypes

import concourse.bacc as bacc
import concourse.tile as tile
from concourse import mybir
from concourse.bass_utils import run_bass_kernel_spmd

N_CORES = 8
B_FULL, C_IN, H, W = 32, 128, 64, 64
C_OUT, KH, KW = 256, 3, 3
B_LOC = B_FULL // N_CORES          # images per core
H_OUT, W_OUT = H - KH + 1, W - KW + 1   # 62, 62
N_HALF = C_OUT // 128              # 2 output-channel halves
ROWS_PER_CHUNK = 8                 # 8 out rows x 62 cols = 496 <= one PSUM bank
X_ROW_PIECES = [(0, 11), (11, 29), (29, 47), (47, 64)]  # DMA pieces (input rows)

_cached = {}


def _build_nc():
    f32 = mybir.dt.float32
    bf16 = mybir.dt.bfloat16
    nc = bacc.Bacc()

    x_d = nc.declare_dram_parameter("x", [B_LOC, C_IN, H, W], bf16, isOutput=False)
    w_d = nc.declare_dram_parameter(
        "w", [C_IN, N_HALF, KH * KW, 128], bf16, isOutput=False
    )
    b_d = nc.declare_dram_parameter("bias_in", [128, N_HALF], f32, isOutput=False)
    y_d = nc.declare_dram_parameter(
        "y", [B_LOC, N_HALF, 128, H_OUT, W_OUT], f32, isOutput=True
    )

    n_chunks = (H_OUT + ROWS_PER_CHUNK - 1) // ROWS_PER_CHUNK
    with tile.TileContext(nc) as tc:
        with (
            tc.tile_pool(name="const", bufs=1) as cpool,
            tc.tile_pool(name="xin", bufs=2) as xpool,
            tc.tile_pool(name="out", bufs=4) as opool,
            tc.tile_pool(name="psum", bufs=6, space="PSUM") as ppool,
        ):
            w_t = cpool.tile([C_IN, N_HALF, KH * KW, 128], bf16)
            b_t = cpool.tile([128, N_HALF], f32)

            # Bias rides the (otherwise idle) ScalarE HWDGE ring.
            nc.scalar.dma_start(b_t[:], b_d[:])
            # Critical path for the first matmul group: w half0 taps 0-2, then
            # x piece0 on the Sync HWDGE ring. The [c, half, uv, o] host layout
            # keeps every tap subrange contiguous per partition, so the first
            # three matmuls can start while the remaining taps stream in.
            nc.sync.dma_start(w_t[:, 0, 0:3], w_d[:, 0, 0:3])

            def load_x(b, first):
                x_t = xpool.tile([C_IN, H, W], bf16, tag="x")
                for k, (r0, r1) in enumerate(X_ROW_PIECES):
                    nc.sync.dma_start(x_t[:, r0:r1, :], x_d[b, :, r0:r1, :])
                    if first and k == 0:
                        nc.sync.dma_start(w_t[:, 0, 3 : KH * KW], w_d[:, 0, 3 : KH * KW])
                        nc.sync.dma_start(w_t[:, 1], w_d[:, 1])
                return x_t

            for b in range(B_LOC):
                x_t = load_x(b, first=(b == 0))
                for chunk in range(n_chunks):
                    i0 = chunk * ROWS_PER_CHUNK
                    r = min(ROWS_PER_CHUNK, H_OUT - i0)
                    for half in range(N_HALF):
                        ps = ppool.tile([128, ROWS_PER_CHUNK, W_OUT], f32, tag="ps")
                        for uv in range(KH * KW):
                            u, v = divmod(uv, KW)
                            nc.tensor.matmul(
                                ps[:, 0:r, :],
                                w_t[:, half, uv, :],
                                x_t[:, i0 + u : i0 + u + r, v : v + W_OUT],
                                start=(uv == 0),
                                stop=(uv == KH * KW - 1),
                            )
                        o_t = opool.tile([128, ROWS_PER_CHUNK, W_OUT], f32, tag="o")
                        nc.vector.tensor_scalar_add(
                            o_t[:, 0:r, :], ps[:, 0:r, :], b_t[:, half : half + 1]
                        )
                        nc.sync.dma_start(
                            y_d[b, half, :, i0 : i0 + r, :], o_t[:, 0:r, :]
                        )

    nc.compile()
    if not nc.is_finalized():
        nc.finalize()
    return nc


def kernel(inputs, weights, bias, profile=False, trace_kwargs=None):
    x_bf = np.ascontiguousarray(inputs).astype(ml_dtypes.bfloat16)
    # [O, C, KH, KW] -> [C, half, KH*KW, o_local]  (lhsT layout: contraction dim
    # on partitions; each half contiguous per partition for fast DMA)
    w_bf = np.ascontiguousarray(
        weights.astype(np.float32)
        .reshape(N_HALF, 128, C_IN, KH * KW)
        .transpose(2, 0, 3, 1)
    ).astype(ml_dtypes.bfloat16)
    # [C_OUT, 1] -> [128, N_HALF] with bias_sb[p, h] = bias[h*128 + p]
    b_t = np.ascontiguousarray(
        bias.astype(np.float32).reshape(N_HALF, 128).T
    )

    if "nc" not in _cached:
        _cached["nc"] = _build_nc()
    nc = _cached["nc"]

    in_maps = [
        {
            "x": x_bf[i * B_LOC : (i + 1) * B_LOC],
            "w": w_bf,
            "bias_in": b_t,
        }
        for i in range(N_CORES)
    ]
    res = run_bass_kernel_spmd(
        nc,
        in_maps,
        list(range(N_CORES)),
        trace=profile,
        **(trace_kwargs or {}),
    )
    _cached["last_result"] = res

    shards = []
    for i in range(N_CORES):
        y = res.results[i]["y"]  # [B_LOC, 2, 128, 62, 62]
        shards.append(y.reshape(B_LOC, C_OUT, H_OUT, W_OUT))
    return np.ascontiguousarray(np.concatenate(shards, axis=0), dtype=np.float32)


# revision 3
# speedup vs baseline: 1.2570x; 1.1542x over previous
"""Trainium2 Bass kernel for 3x3 VALID conv: x[32,128,64,64] * w[256,128,3,3] + bias.

Strategy (v3):
  - Data-parallel over batch: 8 cores x 4 images each; weights/bias replicated.
  - Per core: implicit GEMM in bf16. Contraction dim = C_IN = 128 = partition
    dim. For each filter tap (u,v), accumulate
        psum[o, p] += W[c, o; u,v].T @ x[c, p + u*64 + v]
    over the flattened output grid of 62 rows x 64 cols: a single contiguous
    512-element moving operand per matmul (strided multi-segment APs cost
    ~44ns/MM in AP-walk overhead; contiguous streams at the 213ns floor).
    The last 2 cols of each row are invalid and trimmed on host.
  - bf16 operands: LDWEIGHTS is a separate instruction the PE pulls ahead of
    in-flight matmuls (FWL halves its cost), hiding weight loads under the
    512-cycle matmul stream (fp32r paid ~64ns/MM extra).
  - Head: weights stream on the ScalarE HWDGE ring in parallel with x pieces
    on the Sync ring, so the first matmul group's inputs land ASAP.
  - PSUM -> SBUF evacuation + bias add on VectorE; one output DMA per chunk.
"""

import numpy as np
import ml_dtypes

import concourse.bacc as bacc
import concourse.tile as tile
from concourse import mybir
from concourse.bass_utils import run_bass_kernel_spmd

N_CORES = 8
B_FULL, C_IN, H, W = 32, 128, 64, 64
C_OUT, KH, KW = 256, 3, 3
B_LOC = B_FULL // N_CORES          # images per core
H_OUT, W_OUT = H - KH + 1, W - KW + 1   # 62, 62
N_HALF = C_OUT // 128              # 2 output-channel halves
ROWS_PER_CHUNK = 8                 # 8 out rows x 64 cols = 512 = one PSUM bank
X_PAD = (H_OUT + KH - 1) * W + 128  # padded free size so shifted reads stay in-bounds
X_PIECES = [(0, 640), (640, 1824), (1824, 2976), (2976, H * W)]

_cached = {}


def _build_nc():
    f32 = mybir.dt.float32
    bf16 = mybir.dt.bfloat16
    nc = bacc.Bacc()

    x_d = nc.declare_dram_parameter("x", [B_LOC, C_IN, H, W], bf16, isOutput=False)
    w_d = nc.declare_dram_parameter(
        "w", [C_IN, N_HALF, KH * KW, 128], bf16, isOutput=False
    )
    b_d = nc.declare_dram_parameter("bias_in", [128, N_HALF], f32, isOutput=False)
    y_d = nc.declare_dram_parameter(
        "y", [B_LOC, N_HALF, 128, H_OUT, W], f32, isOutput=True
    )

    n_chunks = (H_OUT + ROWS_PER_CHUNK - 1) // ROWS_PER_CHUNK
    with tile.TileContext(nc) as tc:
        with (
            tc.tile_pool(name="const", bufs=1) as cpool,
            tc.tile_pool(name="xin", bufs=2) as xpool,
            tc.tile_pool(name="out", bufs=4) as opool,
            tc.tile_pool(name="psum", bufs=6, space="PSUM") as ppool,
        ):
            w_t = cpool.tile([C_IN, N_HALF, KH * KW, 128], bf16)
            b_t = cpool.tile([128, N_HALF], f32)

            # Weights + bias ride the ScalarE HWDGE ring in parallel with the
            # x pieces on the Sync ring: tap 0 of half 0 first (unblocks the
            # first LDWEIGHTS), then the rest.
            nc.scalar.dma_start(b_t[:], b_d[:])
            nc.scalar.dma_start(w_t[:, 0, 0:1], w_d[:, 0, 0:1])
            nc.scalar.dma_start(w_t[:, 0, 1 : KH * KW], w_d[:, 0, 1 : KH * KW])
            nc.scalar.dma_start(w_t[:, 1], w_d[:, 1])

            def load_x(b):
                x_t = xpool.tile([C_IN, X_PAD], bf16, tag="x")
                x_flat = x_d[b].rearrange("c h w -> c (h w)")
                for lo, hi in X_PIECES:
                    nc.sync.dma_start(x_t[:, lo:hi], x_flat[:, lo:hi])
                # Tail pad: (arbitrary) real data — feeds only the invalid
                # output columns (j >= 62) that the host trims away.
                nc.sync.dma_start(
                    x_t[:, H * W : X_PAD], x_flat[:, 0 : X_PAD - H * W]
                )
                return x_t

            for b in range(B_LOC):
                x_t = load_x(b)
                for chunk in range(n_chunks):
                    i0 = chunk * ROWS_PER_CHUNK
                    r = min(ROWS_PER_CHUNK, H_OUT - i0)
                    n = r * W
                    p0 = i0 * W
                    for half in range(N_HALF):
                        ps = ppool.tile([128, ROWS_PER_CHUNK, W], f32, tag="ps")
                        for uv in range(KH * KW):
                            u, v = divmod(uv, KW)
                            shift = p0 + u * W + v
                            nc.tensor.matmul(
                                ps[:, 0:r, :],
                                w_t[:, half, uv, :],
                                x_t[:, shift : shift + n],
                                start=(uv == 0),
                                stop=(uv == KH * KW - 1),
                            )
                        o_t = opool.tile([128, ROWS_PER_CHUNK, W], f32, tag="o")
                        nc.vector.tensor_scalar_add(
                            o_t[:, 0:r, :], ps[:, 0:r, :], b_t[:, half : half + 1]
                        )
                        nc.sync.dma_start(
                            y_d[b, half, :, i0 : i0 + r, :], o_t[:, 0:r, :]
                        )

    nc.compile()
    if not nc.is_finalized():
        nc.finalize()
    return nc


def kernel(inputs, weights, bias, profile=False, trace_kwargs=None):
    x_bf = np.ascontiguousarray(inputs).astype(ml_dtypes.bfloat16)
    # [O, C, KH, KW] -> [C, half, KH*KW, o_local]  (lhsT layout: contraction dim
    # on partitions; each half contiguous per partition for fast DMA)
    w_bf = np.ascontiguousarray(
        weights.astype(np.float32)
        .reshape(N_HALF, 128, C_IN, KH * KW)
        .transpose(2, 0, 3, 1)
    ).astype(ml_dtypes.bfloat16)
    # [C_OUT, 1] -> [128, N_HALF] with bias_sb[p, h] = bias[h*128 + p]
    b_t = np.ascontiguousarray(
        bias.astype(np.float32).reshape(N_HALF, 128).T
    )

    if "nc" not in _cached:
        _cached["nc"] = _build_nc()
    nc = _cached["nc"]

    in_maps = [
        {
            "x": x_bf[i * B_LOC : (i + 1) * B_LOC],
            "w": w_bf,
            "bias_in": b_t,
        }
        for i in range(N_CORES)
    ]
    res = run_bass_kernel_spmd(
        nc,
        in_maps,
        list(range(N_CORES)),
        trace=profile,
        **(trace_kwargs or {}),
    )
    _cached["last_result"] = res

    shards = []
    for i in range(N_CORES):
        y = res.results[i]["y"]  # [B_LOC, 2, 128, 62, 64]
        shards.append(y.reshape(B_LOC, C_OUT, H_OUT, W)[..., :W_OUT])
    return np.ascontiguousarray(np.concatenate(shards, axis=0), dtype=np.float32)


# revision 6
# speedup vs baseline: 1.7848x; 1.4198x over previous
"""Trainium2 Bass kernel for 3x3 VALID conv: x[32,128,64,64] * w[256,128,3,3] + bias.

Strategy (v4): 1D Winograd F(2,3) along the width axis.
  - Data-parallel over batch: 8 cores x 4 images each; weights/bias replicated.
  - Host precomputes the input transform V = B^T d per 4-pixel tile (stride 2):
        V0 = x[2t] - x[2t+2];  V1 = x[2t+1] + x[2t+2]
        V2 = x[2t+2] - x[2t+1];  V3 = x[2t+1] - x[2t+3]
    stored [c, xi, row*31] bf16 (t-major, rows adjacent -> contiguous moving
    operands), and the weight transform Gw (per vertical tap u):
        Gw0 = w0;  Gw1 = (w0+w1+w2)/2;  Gw2 = (w0-w1+w2)/2;  Gw3 = w2
  - Device: for each (img, 16-row chunk, half, xi): accumulate 3 matmuls
    (vertical taps) into psum:  M[xi] = sum_u Gw[u,xi]^T @ V[xi, rows+u]
    -> 12 matmuls per chunk-half instead of direct conv's 18 column-streams.
  - Output transform A^T (y_even = M0+M1+M2+b, y_odd = M1-M2-M3+b) is fused
    into PSUM evacuation: ScalarE Identity-copies M1, M2, (M3-b) to bf16;
    VectorE does one fused (M0+b)+C1 scalar_tensor_tensor plus two bf16 2x
    tensor_tensor ops. Even/odd parity blocks stay contiguous; host interleaves.
  - y returned bf16-packed as [b, half, o, parity, i, t]; host upcasts and
    reshapes to [B, 256, 62, 62].
"""

import numpy as np
import ml_dtypes

import concourse.bacc as bacc
import concourse.tile as tile
from concourse import mybir
from concourse.bass_utils import run_bass_kernel_spmd

N_CORES = 8
B_FULL, C_IN, H, W = 32, 128, 64, 64
C_OUT, KH, KW = 256, 3, 3
B_LOC = B_FULL // N_CORES          # images per core
H_OUT, W_OUT = H - KH + 1, W - KW + 1   # 62, 62
N_HALF = C_OUT // 128              # 2 output-channel halves
N_XI = 4                           # F(2,3) m-points
N_T = W_OUT // 2                   # 31 winograd tiles per row
ROWS_PER_CHUNK = 16                # 16 rows x 31 tiles = 496 <= one PSUM bank
CHUNKS = [(0, 16), (16, 16), (32, 16), (48, 14)]
# V row-ranges per DMA piece: chunk c needs V rows [i0, i0+r+1]
V_PIECES = [(0, 18), (18, 34), (34, 50), (50, 64)]

_cached = {}


def _build_nc():
    f32 = mybir.dt.float32
    bf16 = mybir.dt.bfloat16
    add = mybir.AluOpType.add
    sub = mybir.AluOpType.subtract
    ident = mybir.ActivationFunctionType.Identity
    nc = bacc.Bacc()

    v_d = nc.declare_dram_parameter("v", [B_LOC, C_IN, N_XI, H * N_T], bf16,
                                    isOutput=False)
    gw_d = nc.declare_dram_parameter("gw", [C_IN, N_HALF, N_XI, KH, 128], bf16,
                                     isOutput=False)
    # bias columns: [b_h0, b_h1, -b_h0, -b_h1]
    b_d = nc.declare_dram_parameter("bias_in", [128, 2 * N_HALF], f32,
                                    isOutput=False)
    y_d = nc.declare_dram_parameter(
        "y", [B_LOC, N_HALF, 128, 2, H_OUT * N_T], bf16, isOutput=True
    )

    with tile.TileContext(nc) as tc:
        with (
            tc.tile_pool(name="const", bufs=1) as cpool,
            tc.tile_pool(name="vin", bufs=2) as vpool,
            tc.tile_pool(name="mid", bufs=3) as mpool,
            tc.tile_pool(name="out", bufs=4) as opool,
            tc.tile_pool(name="psum", bufs=8, space="PSUM") as ppool,
        ):
            gw_t = cpool.tile([C_IN, N_HALF, N_XI, KH, 128], bf16)
            b_t = cpool.tile([128, 2 * N_HALF], f32)

            # Weights + bias on the ScalarE HWDGE ring, xi=0 of half 0 first
            # (unblocks the first matmul group) in parallel with V piece 0 on
            # the Sync ring.
            nc.scalar.dma_start(b_t[:], b_d[:])
            nc.scalar.dma_start(gw_t[:, 0, 0], gw_d[:, 0, 0])
            nc.scalar.dma_start(gw_t[:, 0, 1:N_XI], gw_d[:, 0, 1:N_XI])
            nc.scalar.dma_start(gw_t[:, 1], gw_d[:, 1])

            def load_v(b):
                v_t = vpool.tile([C_IN, N_XI, H * N_T], bf16, tag="v")
                for r0, r1 in V_PIECES:
                    nc.sync.dma_start(
                        v_t[:, :, r0 * N_T : r1 * N_T],
                        v_d[b, :, :, r0 * N_T : r1 * N_T],
                    )
                return v_t

            for b in range(B_LOC):
                v_t = load_v(b)
                for i0, r in CHUNKS:
                    n = r * N_T
                    for half in range(N_HALF):
                        ps = [None] * N_XI
                        for xi in range(N_XI):
                            ps[xi] = ppool.tile([128, ROWS_PER_CHUNK * N_T],
                                                f32, tag="ps", name=f"ps{xi}")
                            for u in range(KH):
                                lo = (i0 + u) * N_T
                                nc.tensor.matmul(
                                    ps[xi][:, 0:n],
                                    gw_t[:, half, xi, u, :],
                                    v_t[:, xi, lo : lo + n],
                                    start=(u == 0),
                                    stop=(u == KH - 1),
                                )
                        # Output transform A^T + bias, fused into evacuation.
                        c1 = mpool.tile([128, ROWS_PER_CHUNK * N_T], bf16, tag="c1")
                        c2 = mpool.tile([128, ROWS_PER_CHUNK * N_T], bf16, tag="c2")
                        c3 = mpool.tile([128, ROWS_PER_CHUNK * N_T], bf16, tag="c3")
                        te = mpool.tile([128, ROWS_PER_CHUNK * N_T], bf16, tag="te")
                        td = mpool.tile([128, ROWS_PER_CHUNK * N_T], bf16, tag="td")
                        o_t = opool.tile([128, 2, ROWS_PER_CHUNK * N_T], bf16,
                                         tag="o")
                        nc.scalar.activation(c1[:, 0:n], ps[1][:, 0:n], ident)
                        nc.scalar.activation(c2[:, 0:n], ps[2][:, 0:n], ident)
                        # c3 = M3 - b  (bias column 2+half holds -b)
                        nc.scalar.activation(
                            c3[:, 0:n], ps[3][:, 0:n], ident,
                            bias=b_t[:, 2 + half : 3 + half],
                        )
                        # te = (M0 + b) + C1
                        nc.vector.scalar_tensor_tensor(
                            te[:, 0:n], ps[0][:, 0:n],
                            b_t[:, half : half + 1], c1[:, 0:n], add, add,
                        )
                        # y_even = te + C2 ; y_odd = (C1 - C2) - C3
                        nc.vector.tensor_add(o_t[:, 0, 0:n], te[:, 0:n], c2[:, 0:n])
                        nc.vector.tensor_sub(td[:, 0:n], c1[:, 0:n], c2[:, 0:n])
                        nc.vector.tensor_sub(o_t[:, 1, 0:n], td[:, 0:n], c3[:, 0:n])
                        nc.sync.dma_start(
                            y_d[b, half, :, :, i0 * N_T : i0 * N_T + n],
                            o_t[:, :, 0:n],
                        )

    nc.compile()
    if not nc.is_finalized():
        nc.finalize()
    return nc


def kernel(inputs, weights, bias, profile=False, trace_kwargs=None):
    x = np.ascontiguousarray(inputs, dtype=np.float32)
    w = np.ascontiguousarray(weights, dtype=np.float32)

    # Input transform V = B^T d per (row, tile): [B, c, xi, 64*31] bf16
    xe = x[..., 0::2]   # [B, C, 64, 32]
    xo = x[..., 1::2]
    v = np.empty((B_FULL, C_IN, N_XI, H, N_T), dtype=np.float32)
    v[:, :, 0] = xe[..., :N_T] - xe[..., 1 : N_T + 1]
    v[:, :, 1] = xo[..., :N_T] + xe[..., 1 : N_T + 1]
    v[:, :, 2] = xe[..., 1 : N_T + 1] - xo[..., :N_T]
    v[:, :, 3] = xo[..., :N_T] - xo[..., 1 : N_T + 1]
    v_bf = np.ascontiguousarray(
        v.reshape(B_FULL, C_IN, N_XI, H * N_T)
    ).astype(ml_dtypes.bfloat16)

    # Weight transform Gw: [O, C, u, v] -> [c, half, xi, u, o_local] bf16
    g0 = w[..., 0]
    g1 = (w[..., 0] + w[..., 1] + w[..., 2]) * 0.5
    g2 = (w[..., 0] - w[..., 1] + w[..., 2]) * 0.5
    g3 = w[..., 2]
    gw = np.stack([g0, g1, g2, g3], axis=2)     # [O, C, xi, u]
    gw = gw.reshape(N_HALF, 128, C_IN, N_XI, KH).transpose(2, 0, 3, 4, 1)
    gw_bf = np.ascontiguousarray(gw).astype(ml_dtypes.bfloat16)

    bb = bias.astype(np.float32).reshape(N_HALF, 128).T   # [128, half]
    b_t = np.ascontiguousarray(
        np.concatenate([bb, -bb], axis=1)                 # [128, 4]
    )

    if "nc" not in _cached:
        _cached["nc"] = _build_nc()
    nc = _cached["nc"]

    in_maps = [
        {
            "v": v_bf[i * B_LOC : (i + 1) * B_LOC],
            "gw": gw_bf,
            "bias_in": b_t,
        }
        for i in range(N_CORES)
    ]
    res = run_bass_kernel_spmd(
        nc,
        in_maps,
        list(range(N_CORES)),
        trace=profile,
        **(trace_kwargs or {}),
    )
    _cached["last_result"] = res

    shards = []
    for i in range(N_CORES):
        y = res.results[i]["y"]  # [B_LOC, 2, 128, 2, 62*31] bf16
        y = np.asarray(y).astype(np.float32)
        y = y.reshape(B_LOC, C_OUT, 2, H_OUT, N_T)
        # [b, o, parity, i, t] -> [b, o, i, t, parity] -> [b, o, 62, 62]
        y = y.transpose(0, 1, 3, 4, 2).reshape(B_LOC, C_OUT, H_OUT, W_OUT)
        shards.append(y)
    return np.ascontiguousarray(np.concatenate(shards, axis=0), dtype=np.float32)
